# revision 8
# baseline (speedup 1.0000x reference)
"""Trainium2 Bass kernel for nn_LocalLayer (GNN message passing).

out = concat(x, segsum(x[pos_src] -> pos_dst), segsum(x[neg_src] -> neg_dst)) @ W + b

Strategy (8 NeuronCores, no collectives):
  - Shard by destination rows: core c owns output rows [c*12500, (c+1)*12500).
  - Host groups each edge set's edges by (dst window of 128 rows, src phase = src%4),
    giving per-(window, stream) slot groups padded to a fixed quota Q.
  - On device: dma_gather pulls x rows (packed bf16 hi/lo pairs, 512B each) by
    src index (int16, phase trick keeps indices < 25000).
  - Scatter-add realized as one-hot matmuls: S[edge, dst_slot] built on DVE
    (iota vs dst compare), PE accumulates aggT[feat, dst] += X_hi^T@S + X_lo^T@S
    in PSUM per window.
  - Stage 2 per window: out[dst, fout] = xT^T@W0 + posT^T@W1 + negT^T@W2 + b.
"""

import sys
import numpy as np

sys.path.insert(0, "/opt/trn_rl_repo")

import ml_dtypes

from concourse import bass, bacc, mybir, tile
from concourse import bass_utils

F32 = mybir.dt.float32
BF16 = mybir.dt.bfloat16
I16 = mybir.dt.int16

N_NODES = 100000
N_EDGES = 500000
D = 128
D_OUT = 128
N_CORES = 8
CHUNK = N_NODES // N_CORES          # 12500 dst rows per core
WIN = 128                           # dst rows per window
N_WIN = (CHUNK + WIN - 1) // WIN    # 98
W_CH = 7                            # windows per superchunk
N_PHASE = 4
PHASE_ROWS = N_NODES // N_PHASE     # 25000 (< int16 max when idx = src//4)
PAD_DSTV = 999.0
MAX_GATHER_CALL = 1024
N_QUEUES = 1

_cache = {}


# --------------------------------------------------------------------------
# Host-side sharding
# --------------------------------------------------------------------------

def _shard_edges(edge_index):
    """Per (core, phase): slot arrays of gather idx (int16) and dstv (f32).

    Returns (counts, per_core_data) where per_core_data[c][p] =
    (sorted_w, sorted_gidx, sorted_dstv, counts_per_window).
    """
    src = np.asarray(edge_index[0], dtype=np.int64)
    dst = np.asarray(edge_index[1], dtype=np.int64)
    out = []
    maxc = 0
    for c in range(N_CORES):
        sel = (dst >= c * CHUNK) & (dst < (c + 1) * CHUNK)
        s_c = src[sel]
        dl = dst[sel] - c * CHUNK
        w = dl >> 7
        dstv = (dl & 127).astype(np.float32)
        phase = (s_c & 3).astype(np.int64)
        gidx = (s_c >> 2).astype(np.int16)
        per_p = []
        for p in range(N_PHASE):
            m = phase == p
            wp, gp, dp = w[m], gidx[m], dstv[m]
            order = np.argsort(wp, kind="stable")
            wp, gp, dp = wp[order], gp[order], dp[order]
            cnt = np.bincount(wp, minlength=N_WIN).astype(np.int64)
            maxc = max(maxc, int(cnt.max()))
            per_p.append((wp, gp, dp, cnt))
        out.append(per_p)
    return maxc, out


def _fill_slots(per_p, Q):
    """Build flat slot arrays for one (core, phase): gidx int16, dstv f32."""
    wp, gp, dp, cnt = per_p
    starts = np.zeros(N_WIN, np.int64)
    starts[1:] = np.cumsum(cnt)[:-1]
    pos = np.arange(wp.shape[0], dtype=np.int64) - starts[wp]
    slot = wp * Q + pos
    gidx_arr = np.zeros(N_WIN * Q, np.int16)
    dstv_arr = np.full(N_WIN * Q, PAD_DSTV, np.float32)
    gidx_arr[slot] = gp
    dstv_arr[slot] = dp
    return gidx_arr, dstv_arr


def _wrap_idx(gidx_arr, Q, chunks):
    """Wrap the slot-ordered idx array into the dma_gather SBUF layout:
    per superchunk: [16, n/16] (idx i at [i%16, i//16]), replicated to 128
    partitions. Concatenate chunks along columns -> [128, total_cols]."""
    cols = []
    off = 0
    for nw in chunks:
        n = nw * Q
        seg = gidx_arr[off:off + n].reshape(n // 16, 16).T  # [16, n/16]
        cols.append(seg)
        off += n
    a = np.concatenate(cols, axis=1)
    return np.ascontiguousarray(np.tile(a, (8, 1)))


def _prep_core(pos_pp, neg_pp, Q, chunks):
    """Build idx + dstv uploads for one core. Returns dict of arrays."""
    idx_list, dstv_list = [], []
    for per_p in (pos_pp, neg_pp):
        for p in range(N_PHASE):
            g_arr, d_arr = _fill_slots(per_p[p], Q)
            idx_list.append(_wrap_idx(g_arr, Q, chunks))
            # dstv layout: [128, n_tiles], slot t*128+k -> [k, t]
            dstv_list.append(np.ascontiguousarray(
                d_arr.reshape(-1, 128).T))
    return (np.stack(idx_list), np.stack(dstv_list))


# --------------------------------------------------------------------------
# Device program
# --------------------------------------------------------------------------

def _build_program(Q, chunks):
    nt_w = Q // 128                       # gather tiles per (window, stream)
    n_streams = 2 * N_PHASE
    idx_cols = sum(nw * Q for nw in chunks) // 16
    n_tiles_stream = N_WIN * nt_w

    nc = bacc.Bacc("TRN2", target_bir_lowering=False, debug=False,
                   num_devices=N_CORES,
                   dynamic_dma_scratch_size=32768)

    xp_d = nc.dram_tensor("xp", [N_NODES, 2 * D], BF16, kind="ExternalInput")
    xc_d = nc.dram_tensor("xc", [CHUNK, D], F32, kind="ExternalInput")
    w_d = nc.dram_tensor("w", [3 * D, D_OUT], F32, kind="ExternalInput")
    bias_d = nc.dram_tensor("bias", [128, D_OUT], F32, kind="ExternalInput")
    iota_d = nc.dram_tensor("iota", [128, 128], BF16, kind="ExternalInput")
    eye_d = nc.dram_tensor("eye", [128, 128], F32, kind="ExternalInput")
    idx_d = nc.dram_tensor("idx", [n_streams, 128, idx_cols], I16,
                           kind="ExternalInput")
    dstv_d = nc.dram_tensor("dstv", [n_streams, 128, n_tiles_stream], F32,
                            kind="ExternalInput")
    out_d = nc.dram_tensor("out", [CHUNK, D_OUT], F32, kind="ExternalOutput")

    xp_view = xp_d[:, :].rearrange("(a b) c -> a b c", b=N_PHASE)

    with tile.TileContext(nc) as tc:
        with (
            tc.tile_pool(name="consts", bufs=1) as cpool,
            tc.tile_pool(name="gbuf", bufs=2) as gpool,
            tc.tile_pool(name="sbuild", bufs=8) as spool,
            tc.tile_pool(name="aggT", bufs=6) as apool,
            tc.tile_pool(name="outst", bufs=2) as opool,
            tc.tile_pool(name="xw", bufs=3) as xpool,
            tc.tile_pool(name="psA", bufs=4, space="PSUM") as psA,
            tc.tile_pool(name="psX", bufs=2, space="PSUM") as psX,
            tc.tile_pool(name="psO", bufs=2, space="PSUM") as psO,
        ):
            iota_t = cpool.tile([128, 128], BF16, tag="iota")
            eye_t = cpool.tile([128, 128], F32, tag="eye")
            bias_t = cpool.tile([128, D_OUT], F32, tag="bias")
            w_ts = []
            for k in range(3):
                w_t = cpool.tile([128, D_OUT], F32, tag=f"w{k}")
                nc.sync.dma_start(w_t[:], w_d[k * 128:(k + 1) * 128, :])
                w_ts.append(w_t)
            nc.sync.dma_start(iota_t[:], iota_d[:, :])
            nc.sync.dma_start(eye_t[:], eye_d[:, :])
            nc.sync.dma_start(bias_t[:], bias_d[:, :])
            idx_ts = []
            dstv_ts = []
            for st in range(n_streams):
                it = cpool.tile([128, idx_cols], I16, tag=f"idx{st}")
                nc.sync.dma_start(it[:], idx_d[st, :, :])
                idx_ts.append(it)
                dt_ = cpool.tile([128, n_tiles_stream], F32, tag=f"dstv{st}")
                nc.sync.dma_start(dt_[:], dstv_d[st, :, :])
                dstv_ts.append(dt_)

            w_base = 0
            icol_base = 0
            for nw in chunks:
                n_slots = nw * Q
                n_icols = n_slots // 16
                gts = []
                for st in range(n_streams):
                    p = st % N_PHASE
                    gt = gpool.tile([128, nw * nt_w, 2 * D], BF16,
                                    tag=f"g{st}")
                    off = 0
                    while off < n_slots:
                        n_call = min(MAX_GATHER_CALL, n_slots - off)
                        ic0 = icol_base + off // 16
                        nc.gpsimd.dma_gather(
                            gt[:, off // 128:(off + n_call) // 128, :],
                            xp_view[:, p, :],
                            idx_ts[st][:, ic0:ic0 + n_call // 16],
                            n_call, n_call, 2 * D,
                            elem_step=N_PHASE * 2 * D,
                            queue_num=st % N_QUEUES,
                        )
                        off += n_call
                    gts.append(gt)

                out_st = opool.tile([128, nw * D_OUT], F32, tag="out")
                for wl in range(nw):
                    w = w_base + wl
                    aggs = []
                    for s2 in range(2):
                        agg_ps = psA.tile([128, 128], F32, tag="agg")
                        n_mm = N_PHASE * nt_w
                        i_mm = 0
                        for p in range(N_PHASE):
                            st = s2 * N_PHASE + p
                            gt = gts[st]
                            for t2 in range(nt_w):
                                g = wl * nt_w + t2
                                col = (w * nt_w + t2)
                                s_t = spool.tile([128, 128], BF16, tag="S")
                                nc.vector.tensor_scalar(
                                    s_t[:], iota_t[:],
                                    dstv_ts[st][:, col:col + 1], None,
                                    mybir.AluOpType.is_equal)
                                nc.tensor.matmul(
                                    agg_ps[:], gt[:, g, 0:D], s_t[:],
                                    start=(i_mm == 0), stop=False)
                                nc.tensor.matmul(
                                    agg_ps[:], gt[:, g, D:2 * D], s_t[:],
                                    start=False, stop=(i_mm == n_mm - 1))
                                i_mm += 1
                        agg_sb = apool.tile([128, 128], F32, tag="aggT")
                        nc.scalar.copy(agg_sb[:], agg_ps[:])
                        aggs.append(agg_sb)

                    # x term: load own rows, transpose on PE
                    rows = min(WIN, CHUNK - w * WIN)
                    x_t = xpool.tile([128, D], F32, tag="xw")
                    if rows < WIN:
                        nc.vector.memset(x_t[:, :], 0.0)
                    nc.sync.dma_start(x_t[:rows, :],
                                      xc_d[w * WIN:w * WIN + rows, :])
                    xT_ps = psX.tile([128, 128], F32, tag="xT")
                    nc.tensor.transpose(xT_ps[:], x_t[:], eye_t[:])
                    xT_sb = apool.tile([128, 128], F32, tag="xT_sb")
                    nc.scalar.copy(xT_sb[:], xT_ps[:])

                    out_ps = psO.tile([128, D_OUT], F32, tag="outp")
                    nc.tensor.matmul(out_ps[:], xT_sb[:], w_ts[0][:],
                                     start=True, stop=False)
                    nc.tensor.matmul(out_ps[:], aggs[0][:], w_ts[1][:],
                                     start=False, stop=False)
                    nc.tensor.matmul(out_ps[:], aggs[1][:], w_ts[2][:],
                                     start=False, stop=True)
                    nc.vector.tensor_tensor(
                        out_st[:, wl * D_OUT:(wl + 1) * D_OUT],
                        out_ps[:], bias_t[:], mybir.AluOpType.add)

                # store superchunk output
                rows = min(nw * WIN, CHUNK - w_base * WIN)
                dst_ap = out_d[w_base * WIN:w_base * WIN + rows, :]
                if rows == nw * WIN:
                    dst_ap = dst_ap.rearrange("(a p) c -> p a c", p=128)
                    nc.sync.dma_start(dst_ap, out_st[:].rearrange(
                        "p (a c) -> p a c", c=D_OUT))
                else:
                    full = rows // WIN
                    tail = rows - full * WIN
                    if full:
                        nc.sync.dma_start(
                            out_d[w_base * WIN:w_base * WIN + full * WIN, :]
                            .rearrange("(a p) c -> p a c", p=128),
                            out_st[:, :full * D_OUT].rearrange(
                                "p (a c) -> p a c", c=D_OUT))
                    if tail:
                        nc.sync.dma_start(
                            out_d[w_base * WIN + full * WIN:
                                  w_base * WIN + rows, :],
                            out_st[:tail, full * D_OUT:(full + 1) * D_OUT])

                w_base += nw
                icol_base += n_icols

    nc.compile()
    return nc


# --------------------------------------------------------------------------
# Entry point
# --------------------------------------------------------------------------

def kernel(x, pos_edge_index, neg_edge_index, W, b):
    x = np.asarray(x, dtype=np.float32)
    pos_edge_index = np.asarray(pos_edge_index)
    neg_edge_index = np.asarray(neg_edge_index)
    W = np.asarray(W, dtype=np.float32)
    b = np.asarray(b, dtype=np.float32)

    maxc_p, pos_data = _shard_edges(pos_edge_index)
    maxc_n, neg_data = _shard_edges(neg_edge_index)
    Q = max(256, 128 * ((max(maxc_p, maxc_n) + 127) // 128))

    chunks = [W_CH] * (N_WIN // W_CH)
    if N_WIN % W_CH:
        chunks.append(N_WIN % W_CH)

    key = (Q, tuple(chunks))
    if key not in _cache:
        _cache[key] = _build_program(Q, chunks)
    nc = _cache[key]

    # shared uploads
    x_hi = x.astype(ml_dtypes.bfloat16)
    x_lo = (x - x_hi.astype(np.float32)).astype(ml_dtypes.bfloat16)
    xp = np.ascontiguousarray(
        np.concatenate([x_hi, x_lo], axis=1))          # [N, 256] bf16
    bias_rep = np.ascontiguousarray(np.tile(b[None, :], (128, 1)))
    iota = np.ascontiguousarray(
        np.tile(np.arange(128, dtype=np.float32), (128, 1))
    ).astype(ml_dtypes.bfloat16)
    eye = np.eye(128, dtype=np.float32)

    in_maps = []
    for c in range(N_CORES):
        idx_arr, dstv_arr = _prep_core(pos_data[c], neg_data[c], Q, chunks)
        in_maps.append({
            "xp": xp,
            "xc": np.ascontiguousarray(x[c * CHUNK:(c + 1) * CHUNK]),
            "w": W,
            "bias": bias_rep,
            "iota": iota,
            "eye": eye,
            "idx": idx_arr,
            "dstv": dstv_arr,
        })

    res = bass_utils.run_bass_kernel_spmd(
        nc, in_maps, core_ids=list(range(N_CORES)), trace=TRACE,
        **TRACE_KWARGS)
    global LAST_RESULT
    LAST_RESULT = res
    out = np.concatenate([res.results[c]["out"] for c in range(N_CORES)],
                         axis=0)
    return out


TRACE = False
TRACE_KWARGS = {}
LAST_RESULT = None
